# revision 8
# baseline (speedup 1.0000x reference)
"""Sparse multi-head attention (B=4, S=2048, F=512, H=8, D=64) on 8 trn2 cores.

Sharding: core c handles batch b = c % 4 and heads [hg*4, hg*4+4) with
hg = c // 4.  Per-core engine-balanced design:

PE (all matmuls, "scores transposed" layout, head PAIRS):
  - QK projection: W^T-stationary, fp16, contraction row-split into
    (0:64)/(64:128) tile_position pairs so LDWEIGHTS pulls ahead.
    K-bias dropped (softmax-invariant); Q-bias carries the pair scale.
  - V produced directly in [t, d] layout: X^T-block stationary, W_v
    moving, row-split; lands as [t, 4*64] per t-block (no transposes).
  - scores: 2 heads run CONCURRENTLY as k=64 row-tiles into ONE shared
    [128, 2048] psum tile (h0 cols 0:1024, h1 cols 1024:2048).
  - attn*V: lhsT = V_aug [t, 65] (ones col 64 -> denominator row),
    row-split k=64 halves for LDW pull-ahead.
  - V-bias dropped: softmax weights sum to 1 => host adds b_v to Y.

exp is split BY HEAD so the shared scores tile is released after
max(ACT, DVE) instead of a serial chain:
  - h0: ACT exp (scale=1/A) from psum -> bf16, then mask-mult
    (bf16 x {1,0}) on DVE or GPSIMD (alternating tbs).
  - h1: single fused DVE op: i16 = convert(psum + mask_i16[t,s]);
    psum = A*score with A = 128*log2(e) folded into Wq/Wk on host, so
    the i16 IS the bf16 bit pattern of exp(score) (Schraudolph), and
    mask_i16 in {16249, -32768} does exp+mask in one op (masked lands
    at tiny-negative bf16 ~ -0).  ~0.8% extra rel err (measured).

Host divides by the denominator row and interleaves heads.
"""

import sys

for _p in ("/opt/trn_rl_repo", "/root/.axon_site/_ro/trn_rl_repo"):
    if _p not in sys.path:
        sys.path.insert(0, _p)

from contextlib import ExitStack

import ml_dtypes
import numpy as np

import concourse.bacc as bacc
import concourse.tile as tile
from concourse import bass_utils, mybir

B, S, F, H, D = 4, 2048, 512, 8, 64
HPC = H // 2  # heads per core (4): 2 head-groups x 4 batches = 8 cores
NPAIR = HPC // 2  # head pairs per core (2)
N_CORES = 8
NF = F // 128  # 4 f-chunks of 128
NT = S // 128  # 16 t-blocks

A_SCALE = 128.0 * np.log2(np.e)  # psum = A_SCALE * score
C_UNMASK = 16249  # 16256 + delta, delta = -7 (var-min ~ -7.3)
C_MASK = -32768

F32 = mybir.dt.float32
BF16 = mybir.dt.bfloat16
FP16 = mybir.dt.float16
I16 = mybir.dt.int16
AF = mybir.ActivationFunctionType
ALU = mybir.AluOpType


def build_nc():
    nc = bacc.Bacc(
        "TRN2", target_bir_lowering=False, debug=False, num_devices=N_CORES
    )
    xt_d = nc.dram_tensor("xt", [F, S], FP16, kind="ExternalInput").ap()
    mkb_d = nc.dram_tensor("mkb", [S, S], I16, kind="ExternalInput").ap()
    mki_d = nc.dram_tensor("mki", [S, S], I16, kind="ExternalInput").ap()
    wqk_d = nc.dram_tensor("wqk", [F, NPAIR * 256], FP16, kind="ExternalInput").ap()
    wv_d = nc.dram_tensor("wv", [F, HPC * 64], FP16, kind="ExternalInput").ap()
    bq_d = nc.dram_tensor("bq", [128, NPAIR], F32, kind="ExternalInput").ap()
    yt_d = nc.dram_tensor("yt", [HPC, 65, S], F32, kind="ExternalOutput").ap()
    junk_d = nc.dram_tensor("junk", [64, 512], F32)  # warmup sink (Internal)

    with ExitStack() as ctx:
        tc = ctx.enter_context(tile.TileContext(nc))
        const = ctx.enter_context(tc.tile_pool(name="const", bufs=1))

        wqk_sb = const.tile([128, NF, NPAIR * 256], FP16)
        nc.sync.dma_start(wqk_sb[:], wqk_d.rearrange("(c p) n -> p c n", p=128))
        bq_sb = const.tile([128, NPAIR], F32)
        nc.sync.dma_start(bq_sb[:], bq_d)

        xt_sb = const.tile([128, NF, S], FP16)
        xt_r = xt_d.rearrange("(c p) s -> p c s", p=128)
        wv_sb = const.tile([128, NF, HPC * 64], FP16)
        for sh in range(2):
            hsl = slice(sh * (S // 2), (sh + 1) * (S // 2))
            for c in range(NF):
                nc.sync.dma_start(xt_sb[:, c, hsl], xt_r[:, c, hsl])
            if sh == 0:
                nc.sync.dma_start(
                    wv_sb[:], wv_d.rearrange("(c p) n -> p c n", p=128)
                )
        mkb_sb = const.tile([128, NT, S], I16)
        mki_sb = const.tile([128, NT, S], I16)
        mkb_r = mkb_d.rearrange("(t p) s -> p t s", p=128)
        mki_r = mki_d.rearrange("(t p) s -> p t s", p=128)
        for i in range(8):
            g = NT // 8
            sl = slice(i * g, (i + 1) * g)
            nc.sync.dma_start(mkb_sb[:, sl, :], mkb_r[:, sl, :])
            nc.sync.dma_start(mki_sb[:, sl, :], mki_r[:, sl, :])

        # V_aug in [t, 65] layout per (head, t-block); col 64 = ones
        v_sb = const.tile([128, HPC, NT, 65], BF16)
        nc.vector.memset(v_sb[:, :, :, 64:65], 1.0)
        qt_sb = [const.tile([128, S], FP16, name=f"qt{m}") for m in range(NPAIR)]
        kt_sb = [const.tile([128, S], FP16, name=f"kt{m}") for m in range(NPAIR)]

        e_pool = ctx.enter_context(tc.tile_pool(name="e", bufs=2))
        y_pool = ctx.enter_context(tc.tile_pool(name="y", bufs=1))
        ps = ctx.enter_context(tc.tile_pool(name="ps", bufs=1, space="PSUM"))

        # --- PE warmup: open the HAM clock gate (~4us of matmuls) while
        # the input DMAs land.
        NWU = 12
        wu = const.tile([128, 512], BF16)
        nc.vector.memset(wu[:], 0.0)
        pw = ps.tile([128, 512], F32, tag="sc", name="pw")
        for i in range(NWU):
            nc.tensor.matmul(
                pw[:], wu[:, 0:128], wu[:], start=(i == 0), stop=(i == NWU - 1)
            )
        wu_out = const.tile([64, 512], F32)
        nc.vector.tensor_copy(wu_out[:], pw[0:64, :])
        nc.sync.dma_start(junk_d.ap(), wu_out[:])

        # --- Phase 0a: QK projection (row-split c-chunks; 3-deep psum
        # rotation over the attention tags).
        tags = ("sc", "ya", "yb")
        ti = 0
        for m in range(NPAIR):
            for sq in range(S // 512):
                ssl = slice(sq * 512, (sq + 1) * 512)
                for kind in range(2):  # 0 = q, 1 = k
                    wsl = slice(m * 256 + kind * 128, m * 256 + (kind + 1) * 128)
                    pp = ps.tile([128, 512], F32, tag=tags[ti % 3], name="pp")
                    ti += 1
                    for c in range(NF):
                        nc.tensor.matmul(
                            pp[:],
                            wqk_sb[:, c, wsl],
                            xt_sb[:, c, ssl],
                            start=(c == 0),
                            stop=(c == NF - 1),
                        )
                    if kind == 0:
                        nc.vector.tensor_scalar(
                            qt_sb[m][:, ssl],
                            pp[:],
                            bq_sb[:, m : m + 1],
                            None,
                            op0=ALU.add,
                        )
                    else:
                        nc.scalar.activation(kt_sb[m][:, ssl], pp[:], AF.Copy)

        # --- Phase 0b: V via X^T-stationary, row-split; [t, d] layout.
        for tb in range(NT):
            tsl = slice(tb * 128, (tb + 1) * 128)
            vp = ps.tile([128, 512], F32, tag=tags[ti % 3], name="vp")
            ti += 1
            for c in range(NF):
                nc.tensor.matmul(
                    vp[:, 0 : HPC * 64],
                    xt_sb[:, c, tsl],
                    wv_sb[:, c, :],
                    start=(c == 0),
                    stop=(c == NF - 1),
                )
            # [128, 4, 64] strided dst: per-head 64-col V blocks
            nc.scalar.activation(
                v_sb[:, :, tb, 0:64], vp[:, 0 : HPC * 64], AF.Copy
            )

        # --- Attention: per (pair, query-half of 1024).  Scores for both
        # heads land in ONE shared psum tile; h0 -> ACT exp + mask-mult,
        # h1 -> fused DVE Schraudolph exp+mask.  Software-pipelined.
        for m in range(NPAIR):
            qt, kt = qt_sb[m], kt_sb[m]
            for qh in range(2):
                py = [
                    ps.tile([65, 1024], F32, tag=t, name=f"py_{t}")
                    for t in ("ya", "yb")
                ]
                sc = {}

                def emit_scores(tb):
                    tsl = slice(tb * 128, (tb + 1) * 128)
                    p2 = ps.tile([128, 2048], F32, tag="sc", name="p2")
                    for qb in range(2):
                        qsl = slice(qh * 1024 + qb * 512, qh * 1024 + (qb + 1) * 512)
                        nc.tensor.matmul(
                            p2[:, qb * 512 : (qb + 1) * 512],
                            kt[0:64, tsl],
                            qt[0:64, qsl],
                            start=True,
                            stop=True,
                        )
                        nc.tensor.matmul(
                            p2[:, 1024 + qb * 512 : 1024 + (qb + 1) * 512],
                            kt[64:128, tsl],
                            qt[64:128, qsl],
                            start=True,
                            stop=True,
                        )
                    sc[tb] = p2

                emit_scores(0)
                for tb in range(NT):
                    if tb + 1 < NT:
                        emit_scores(tb + 1)
                    p2 = sc.pop(tb)
                    msl = slice(qh * 1024, (qh + 1) * 1024)
                    e0 = e_pool.tile([128, 1024], BF16, tag="e0", name="e0")
                    e1 = e_pool.tile([128, 1024], BF16, tag="e1", name="e1")
                    nc.scalar.activation(
                        e0[:], p2[:, 0:1024], AF.Exp, scale=1.0 / A_SCALE
                    )
                    eng = nc.gpsimd if tb % 2 == 0 else nc.vector
                    eng.tensor_tensor(
                        e0[:],
                        e0[:],
                        mkb_sb[:, tb, msl].bitcast(BF16),
                        op=ALU.mult,
                    )
                    nc.vector.tensor_tensor(
                        e1[:].bitcast(I16),
                        p2[:, 1024:2048],
                        mki_sb[:, tb, msl],
                        op=ALU.add,
                    )
                    for j, e in ((0, e0), (1, e1)):
                        h = 2 * m + j
                        for qb in range(2):
                            osl = slice(qb * 512, (qb + 1) * 512)
                            nc.tensor.matmul(
                                py[j][:, osl],
                                v_sb[:, h, tb, :],
                                e[:, osl],
                                start=(tb == 0),
                                stop=(tb == NT - 1),
                            )
                for j in range(2):
                    y_sb = y_pool.tile([65, 1024], F32, tag=f"y{j}", name="y_sb")
                    nc.scalar.activation(y_sb[:], py[j][:], AF.Copy)
                    nc.sync.dma_start(
                        yt_d[2 * m + j, :, qh * 1024 : (qh + 1) * 1024], y_sb[:]
                    )

    nc.compile()
    return nc


_NC_CACHE = {}


def _get_nc():
    if "nc" not in _NC_CACHE:
        _NC_CACHE["nc"] = build_nc()
    return _NC_CACHE["nc"]


def make_in_maps(X, A, W, b):
    X = np.ascontiguousarray(np.asarray(X), dtype=np.float32)
    A = np.asarray(A)
    W = np.ascontiguousarray(np.asarray(W), dtype=np.float32)
    b = np.ascontiguousarray(np.asarray(b), dtype=np.float32)
    _NC_CACHE["b"] = b
    alpha = np.float32(np.sqrt(A_SCALE / np.sqrt(np.float32(H))))
    d = np.arange(D)

    xts = [np.ascontiguousarray(X[bb].T).astype(np.float16) for bb in range(B)]
    one_bits = np.float32(1.0).astype(ml_dtypes.bfloat16).view(np.int16)
    mkbs, mkis = [], []
    for bb in range(B):
        mT = np.ascontiguousarray(A[bb].T)
        mkbs.append(np.where(mT, one_bits, np.int16(0)).astype(np.int16))
        mkis.append(
            np.where(mT, np.int16(C_UNMASK), np.int16(C_MASK)).astype(np.int16)
        )

    packs = []
    for hg in range(2):
        wqk = np.empty((F, NPAIR * 256), np.float32)
        wv = np.empty((F, HPC * 64), np.float32)
        bq = np.empty((128, NPAIR), np.float32)
        for m in range(NPAIR):
            for half in range(2):
                h = hg * HPC + 2 * m + half
                qc = d * 24 + h
                kc = d * 24 + 8 + h
                vc = d * 24 + 16 + h
                c0 = m * 256 + half * 64
                wqk[:, c0 : c0 + 64] = W[:, qc] * alpha
                wqk[:, c0 + 128 : c0 + 192] = W[:, kc] * alpha
                wv[:, (2 * m + half) * 64 : (2 * m + half + 1) * 64] = W[:, vc]
                rsl = slice(64 * half, 64 * (half + 1))
                bq[rsl, m] = b[qc] * alpha
        packs.append((wqk.astype(np.float16), wv.astype(np.float16), bq))

    in_maps = []
    for c in range(N_CORES):
        bb = c % B
        hg = c // B
        wqk, wv, bq = packs[hg]
        in_maps.append(
            {
                "xt": xts[bb],
                "mkb": mkbs[bb],
                "mki": mkis[bb],
                "wqk": wqk,
                "wv": wv,
                "bq": bq,
            }
        )
    return in_maps


def assemble_output(results):
    b_full = _NC_CACHE["b"]
    Y = np.empty((B, S, D * H), np.float32)
    Yv = Y.reshape(B, S, D, H)
    d = np.arange(D)
    for c in range(N_CORES):
        bb = c % B
        hg = c // B
        yt = results[c]["yt"]  # [HPC, 65, S]
        for j in range(HPC):
            h = hg * HPC + j
            bv = b_full[d * 24 + 16 + h]
            Yv[bb, :, :, h] = (yt[j, 0:64, :] / yt[j, 64:65, :]).T + bv[None, :]
    return Y


def kernel(X, A, W, b):
    nc = _get_nc()
    in_maps = make_in_maps(X, A, W, b)
    res = bass_utils.run_bass_kernel_spmd(
        nc, in_maps, core_ids=list(range(N_CORES))
    ).results
    return assemble_output(res)


# revision 14
# speedup vs baseline: 1.5037x; 1.5037x over previous
"""Sparse multi-head attention (B=4, S=2048, F=512, H=8, D=64) on 8 trn2 cores.

Sharding: core c handles batch b = c % 4 and heads [hg*4, hg*4+4) with
hg = c // 4.  Per-core engine-balanced design:

PE (all matmuls, "scores transposed" layout, head PAIRS):
  - QK projection: W^T-stationary, fp16, contraction row-split into
    (0:64)/(64:128) tile_position pairs so LDWEIGHTS pulls ahead.
    K-bias dropped (softmax-invariant); Q-bias carries the pair scale.
  - V produced directly in [t, d] layout: X^T-block stationary, W_v
    moving, row-split; lands as [t, 4*64] per t-block (no transposes).
  - scores: 2 heads run CONCURRENTLY as k=64 row-tiles into ONE shared
    [128, 2048] psum tile (h0 cols 0:1024, h1 cols 1024:2048).
  - attn*V: lhsT = V_aug [t, 65] (ones col 64 -> denominator row),
    row-split k=64 halves for LDW pull-ahead.
  - V-bias dropped: softmax weights sum to 1 => host adds b_v to Y.

exp is split BY HEAD so the shared scores tile is released after
max(ACT, DVE) instead of a serial chain:
  - h0: ACT exp (scale=1/A) from psum -> bf16, then mask-mult
    (bf16 x {1,0}) on DVE or GPSIMD (alternating tbs).
  - h1: single fused DVE op: i16 = convert(psum + mask_i16[t,s]);
    psum = A*score with A = 128*log2(e) folded into Wq/Wk on host, so
    the i16 IS the bf16 bit pattern of exp(score) (Schraudolph), and
    mask_i16 in {16249, -32768} does exp+mask in one op (masked lands
    at tiny-negative bf16 ~ -0).  ~0.8% extra rel err (measured).

Host divides by the denominator row and interleaves heads.
"""

import sys

for _p in ("/opt/trn_rl_repo", "/root/.axon_site/_ro/trn_rl_repo"):
    if _p not in sys.path:
        sys.path.insert(0, _p)

from contextlib import ExitStack

import ml_dtypes
import numpy as np

import concourse.bacc as bacc
import concourse.tile as tile
from concourse import bass_utils, mybir

B, S, F, H, D = 4, 2048, 512, 8, 64
HPC = H // 2  # heads per core (4): 2 head-groups x 4 batches = 8 cores
NPAIR = HPC // 2  # head pairs per core (2)
N_CORES = 8
NF = F // 128  # 4 f-chunks of 128
NT = S // 128  # 16 t-blocks

A_SCALE = 128.0 * np.log2(np.e)  # psum = A_SCALE * score
C_UNMASK = 16249  # 16256 + delta, delta = -7 (var-min ~ -7.3)
C_MASK = -32768

F32 = mybir.dt.float32
BF16 = mybir.dt.bfloat16
FP16 = mybir.dt.float16
I16 = mybir.dt.int16
AF = mybir.ActivationFunctionType
ALU = mybir.AluOpType


def build_nc():
    nc = bacc.Bacc(
        "TRN2", target_bir_lowering=False, debug=False, num_devices=N_CORES
    )
    xt_d = nc.dram_tensor("xt", [F, S], FP16, kind="ExternalInput").ap()
    mkb_d = nc.dram_tensor("mkb", [S, S], I16, kind="ExternalInput").ap()
    mki_d = nc.dram_tensor("mki", [S, S], I16, kind="ExternalInput").ap()
    wqk_d = nc.dram_tensor("wqk", [F, NPAIR * 256], FP16, kind="ExternalInput").ap()
    wv_d = nc.dram_tensor("wv", [F, HPC * 64], FP16, kind="ExternalInput").ap()
    bq_d = nc.dram_tensor("bq", [128, NPAIR], F32, kind="ExternalInput").ap()
    yt_d = nc.dram_tensor("yt", [HPC, 65, S], F32, kind="ExternalOutput").ap()
    junk_d = nc.dram_tensor("junk", [64, 512], F32)  # warmup sink (Internal)

    with ExitStack() as ctx:
        tc = ctx.enter_context(tile.TileContext(nc))
        const = ctx.enter_context(tc.tile_pool(name="const", bufs=1))

        wqk_sb = const.tile([128, NF, NPAIR * 256], FP16)
        nc.sync.dma_start(wqk_sb[:], wqk_d.rearrange("(c p) n -> p c n", p=128))
        bq_sb = const.tile([128, NPAIR], F32)
        nc.sync.dma_start(bq_sb[:], bq_d)

        xt_sb = const.tile([128, NF, S], FP16)
        xt_r = xt_d.rearrange("(c p) s -> p c s", p=128)
        wv_sb = const.tile([128, NF, HPC * 64], FP16)
        for sh in range(2):
            hsl = slice(sh * (S // 2), (sh + 1) * (S // 2))
            for c in range(NF):
                nc.sync.dma_start(xt_sb[:, c, hsl], xt_r[:, c, hsl])
            if sh == 0:
                nc.sync.dma_start(
                    wv_sb[:], wv_d.rearrange("(c p) n -> p c n", p=128)
                )
        mkb_sb = const.tile([128, NT, S], I16)
        mki_sb = const.tile([128, NT, S], I16)
        mkb_r = mkb_d.rearrange("(t p) s -> p t s", p=128)
        mki_r = mki_d.rearrange("(t p) s -> p t s", p=128)
        for i in range(8):
            g = NT // 8
            sl = slice(i * g, (i + 1) * g)
            nc.sync.dma_start(mkb_sb[:, sl, :], mkb_r[:, sl, :])
            nc.sync.dma_start(mki_sb[:, sl, :], mki_r[:, sl, :])

        # V_aug in [t, 65] layout per (head, t-block); col 64 = ones
        v_sb = const.tile([128, HPC, NT, 65], BF16)
        nc.vector.memset(v_sb[:, :, :, 64:65], 1.0)
        qt_sb = [const.tile([128, S], FP16, name=f"qt{m}") for m in range(NPAIR)]
        kt_sb = [const.tile([128, S], FP16, name=f"kt{m}") for m in range(NPAIR)]

        e_pool = ctx.enter_context(tc.tile_pool(name="e", bufs=4))
        y_pool = ctx.enter_context(tc.tile_pool(name="y", bufs=2))
        ps = ctx.enter_context(tc.tile_pool(name="ps", bufs=1, space="PSUM"))

        # --- PE warmup: open the HAM clock gate (~4us of matmuls) while
        # the input DMAs land.
        NWU = 12
        wu = const.tile([128, 512], BF16)
        nc.vector.memset(wu[:], 0.0)
        pw = ps.tile([128, 512], F32, tag="pp", bufs=2, name="pw")
        for i in range(NWU):
            nc.tensor.matmul(
                pw[:], wu[:, 0:128], wu[:], start=(i == 0), stop=(i == NWU - 1)
            )
        wu_out = const.tile([64, 512], F32)
        nc.vector.tensor_copy(wu_out[:], pw[0:64, :])
        nc.sync.dma_start(junk_d.ap(), wu_out[:])

        # --- Phase 0a: QK projection (double-buffered "pp" psum tag).
        for m in range(NPAIR):
            for sq in range(S // 512):
                ssl = slice(sq * 512, (sq + 1) * 512)
                for kind in range(2):  # 0 = q, 1 = k
                    wsl = slice(m * 256 + kind * 128, m * 256 + (kind + 1) * 128)
                    pp = ps.tile([128, 512], F32, tag="pp", bufs=2, name="pp")
                    for c in range(NF):
                        nc.tensor.matmul(
                            pp[:],
                            wqk_sb[:, c, wsl],
                            xt_sb[:, c, ssl],
                            start=(c == 0),
                            stop=(c == NF - 1),
                        )
                    if kind == 0:
                        nc.vector.tensor_scalar(
                            qt_sb[m][:, ssl],
                            pp[:],
                            bq_sb[:, m : m + 1],
                            None,
                            op0=ALU.add,
                        )
                    else:
                        nc.scalar.activation(kt_sb[m][:, ssl], pp[:], AF.Copy)

        # --- Phase 0b: V via X^T-stationary, row-split; [t, d] layout.
        for tb in range(NT):
            tsl = slice(tb * 128, (tb + 1) * 128)
            vp = ps.tile([128, 512], F32, tag="pp", bufs=2, name="vp")
            for c in range(NF):
                nc.tensor.matmul(
                    vp[:, 0 : HPC * 64],
                    xt_sb[:, c, tsl],
                    wv_sb[:, c, :],
                    start=(c == 0),
                    stop=(c == NF - 1),
                )
            # [128, 4, 64] strided dst: per-head 64-col V blocks
            nc.scalar.activation(
                v_sb[:, :, tb, 0:64], vp[:, 0 : HPC * 64], AF.Copy
            )

        # --- Attention: per (pair, query-block of 512).  Per t-block one
        # shared scores tile [128, 1024] (h0 cols 0:512, h1 cols
        # 512:1024) so the two k=64 row-tile matmuls stay adjacent and
        # overlap in the PE array.  h0 -> ACT exp then DVE mask-mult;
        # h1 -> fused single DVE op (Schraudolph exp+mask), except
        # H1_ACT_TBS where h1 also takes the ACT path (engine balance).
        # 4-stage software pipeline: scores(t+2) / exp(t+1) / mask(t) /
        # attn*V(t-1), so every instruction finds its deps long done and
        # the PE never idles (keeps the HAM clock-gate open).
        H1_ACT_TBS = frozenset((3, 7, 11, 15))
        for m in range(NPAIR):
            qt, kt = qt_sb[m], kt_sb[m]
            for qb in range(4):
                qsl = slice(qb * 512, (qb + 1) * 512)
                py = [
                    ps.tile([65, 512], F32, tag=t, name=f"py_{t}")
                    for t in ("ya", "yb")
                ]
                sc, es = {}, {}

                def emit_scores(tb):
                    tsl = slice(tb * 128, (tb + 1) * 128)
                    p2 = ps.tile([128, 1024], F32, tag="sc", bufs=2, name="p2")
                    nc.tensor.matmul(
                        p2[:, 0:512], kt[0:64, tsl], qt[0:64, qsl],
                        start=True, stop=True,
                    )
                    nc.tensor.matmul(
                        p2[:, 512:1024], kt[64:128, tsl], qt[64:128, qsl],
                        start=True, stop=True,
                    )
                    sc[tb] = p2

                def emit_exp(tb):
                    p2 = sc.pop(tb)
                    e0 = e_pool.tile([128, 512], BF16, tag="e0", name="e0")
                    e1 = e_pool.tile([128, 512], BF16, tag="e1", name="e1")
                    nc.scalar.activation(
                        e0[:], p2[:, 0:512], AF.Exp, scale=1.0 / A_SCALE
                    )
                    if tb in H1_ACT_TBS:
                        nc.scalar.activation(
                            e1[:], p2[:, 512:1024], AF.Exp, scale=1.0 / A_SCALE
                        )
                    else:
                        nc.vector.tensor_tensor(
                            e1[:].bitcast(I16),
                            p2[:, 512:1024],
                            mki_sb[:, tb, qsl],
                            op=ALU.add,
                        )
                    es[tb] = (e0, e1)

                def emit_mask(tb):
                    e0, e1 = es[tb]
                    nc.vector.tensor_tensor(
                        e0[:], e0[:], mkb_sb[:, tb, qsl].bitcast(BF16),
                        op=ALU.mult,
                    )
                    if tb in H1_ACT_TBS:
                        nc.vector.tensor_tensor(
                            e1[:], e1[:], mkb_sb[:, tb, qsl].bitcast(BF16),
                            op=ALU.mult,
                        )

                def emit_av(tb):
                    e0, e1 = es.pop(tb)
                    for j, e in ((0, e0), (1, e1)):
                        nc.tensor.matmul(
                            py[j][:],
                            v_sb[:, 2 * m + j, tb, :],
                            e[:],
                            start=(tb == 0),
                            stop=(tb == NT - 1),
                        )

                emit_scores(0)
                emit_scores(1)
                emit_exp(0)
                for tb in range(NT):
                    if tb + 2 < NT:
                        emit_scores(tb + 2)
                    if tb + 1 < NT:
                        emit_exp(tb + 1)
                    emit_mask(tb)
                    if tb >= 1:
                        emit_av(tb - 1)
                emit_av(NT - 1)
                for j in range(2):
                    y_sb = y_pool.tile([65, 512], F32, tag=f"y{j}", name="y_sb")
                    nc.scalar.activation(y_sb[:], py[j][:], AF.Copy)
                    nc.sync.dma_start(
                        yt_d[2 * m + j, :, qsl], y_sb[:]
                    )

    nc.compile()
    return nc


_NC_CACHE = {}


def _get_nc():
    if "nc" not in _NC_CACHE:
        _NC_CACHE["nc"] = build_nc()
    return _NC_CACHE["nc"]


def make_in_maps(X, A, W, b):
    X = np.ascontiguousarray(np.asarray(X), dtype=np.float32)
    A = np.asarray(A)
    W = np.ascontiguousarray(np.asarray(W), dtype=np.float32)
    b = np.ascontiguousarray(np.asarray(b), dtype=np.float32)
    _NC_CACHE["b"] = b
    alpha = np.float32(np.sqrt(A_SCALE / np.sqrt(np.float32(H))))
    d = np.arange(D)

    xts = [np.ascontiguousarray(X[bb].T).astype(np.float16) for bb in range(B)]
    one_bits = np.float32(1.0).astype(ml_dtypes.bfloat16).view(np.int16)
    mkbs, mkis = [], []
    for bb in range(B):
        mT = np.ascontiguousarray(A[bb].T)
        mkbs.append(np.where(mT, one_bits, np.int16(0)).astype(np.int16))
        mkis.append(
            np.where(mT, np.int16(C_UNMASK), np.int16(C_MASK)).astype(np.int16)
        )

    packs = []
    for hg in range(2):
        wqk = np.empty((F, NPAIR * 256), np.float32)
        wv = np.empty((F, HPC * 64), np.float32)
        bq = np.empty((128, NPAIR), np.float32)
        for m in range(NPAIR):
            for half in range(2):
                h = hg * HPC + 2 * m + half
                qc = d * 24 + h
                kc = d * 24 + 8 + h
                vc = d * 24 + 16 + h
                c0 = m * 256 + half * 64
                wqk[:, c0 : c0 + 64] = W[:, qc] * alpha
                wqk[:, c0 + 128 : c0 + 192] = W[:, kc] * alpha
                wv[:, (2 * m + half) * 64 : (2 * m + half + 1) * 64] = W[:, vc]
                rsl = slice(64 * half, 64 * (half + 1))
                bq[rsl, m] = b[qc] * alpha
        packs.append((wqk.astype(np.float16), wv.astype(np.float16), bq))

    in_maps = []
    for c in range(N_CORES):
        bb = c % B
        hg = c // B
        wqk, wv, bq = packs[hg]
        in_maps.append(
            {
                "xt": xts[bb],
                "mkb": mkbs[bb],
                "mki": mkis[bb],
                "wqk": wqk,
                "wv": wv,
                "bq": bq,
            }
        )
    return in_maps


def assemble_output(results):
    b_full = _NC_CACHE["b"]
    Y = np.empty((B, S, D * H), np.float32)
    Yv = Y.reshape(B, S, D, H)
    d = np.arange(D)
    for c in range(N_CORES):
        bb = c % B
        hg = c // B
        yt = results[c]["yt"]  # [HPC, 65, S]
        for j in range(HPC):
            h = hg * HPC + j
            bv = b_full[d * 24 + 16 + h]
            Yv[bb, :, :, h] = (yt[j, 0:64, :] / yt[j, 64:65, :]).T + bv[None, :]
    return Y


def kernel(X, A, W, b):
    nc = _get_nc()
    in_maps = make_in_maps(X, A, W, b)
    res = bass_utils.run_bass_kernel_spmd(
        nc, in_maps, core_ids=list(range(N_CORES))
    ).results
    return assemble_output(res)


# revision 15
# speedup vs baseline: 1.7348x; 1.1537x over previous
"""Sparse multi-head attention (B=4, S=2048, F=512, H=8, D=64) on 8 trn2 cores.

Sharding: core c handles batch b = c % 4 and heads [hg*4, hg*4+4) with
hg = c // 4.  Per-core engine-balanced design:

PE (all matmuls, "scores transposed" layout, head PAIRS):
  - QK projection: W^T-stationary, fp16, contraction row-split into
    (0:64)/(64:128) tile_position pairs so LDWEIGHTS pulls ahead.
    K-bias dropped (softmax-invariant); Q-bias carries the pair scale.
  - V produced directly in [t, d] layout: X^T-block stationary, W_v
    moving, row-split; lands as [t, 4*64] per t-block (no transposes).
  - scores: 2 heads run CONCURRENTLY as k=64 row-tiles into ONE shared
    [128, 2048] psum tile (h0 cols 0:1024, h1 cols 1024:2048).
  - attn*V: lhsT = V_aug [t, 65] (ones col 64 -> denominator row),
    row-split k=64 halves for LDW pull-ahead.
  - V-bias dropped: softmax weights sum to 1 => host adds b_v to Y.

exp is split BY HEAD so the shared scores tile is released after
max(ACT, DVE) instead of a serial chain:
  - h0: ACT exp (scale=1/A) from psum -> bf16, then mask-mult
    (bf16 x {1,0}) on DVE or GPSIMD (alternating tbs).
  - h1: single fused DVE op: i16 = convert(psum + mask_i16[t,s]);
    psum = A*score with A = 128*log2(e) folded into Wq/Wk on host, so
    the i16 IS the bf16 bit pattern of exp(score) (Schraudolph), and
    mask_i16 in {16249, -32768} does exp+mask in one op (masked lands
    at tiny-negative bf16 ~ -0).  ~0.8% extra rel err (measured).

Host divides by the denominator row and interleaves heads.
"""

import sys

for _p in ("/opt/trn_rl_repo", "/root/.axon_site/_ro/trn_rl_repo"):
    if _p not in sys.path:
        sys.path.insert(0, _p)

from contextlib import ExitStack

import ml_dtypes
import numpy as np

import concourse.bacc as bacc
import concourse.tile as tile
from concourse import bass_utils, mybir

B, S, F, H, D = 4, 2048, 512, 8, 64
HPC = H // 2  # heads per core (4): 2 head-groups x 4 batches = 8 cores
NPAIR = HPC // 2  # head pairs per core (2)
N_CORES = 8
NF = F // 128  # 4 f-chunks of 128
NT = S // 128  # 16 t-blocks

A_SCALE = 128.0 * np.log2(np.e)  # psum = A_SCALE * score
C_UNMASK = 16249  # 16256 + delta, delta = -7 (var-min ~ -7.3)
C_MASK = -32768

F32 = mybir.dt.float32
BF16 = mybir.dt.bfloat16
FP16 = mybir.dt.float16
I16 = mybir.dt.int16
AF = mybir.ActivationFunctionType
ALU = mybir.AluOpType


def build_nc():
    nc = bacc.Bacc(
        "TRN2", target_bir_lowering=False, debug=False, num_devices=N_CORES
    )
    xt_d = nc.dram_tensor("xt", [F, S], FP16, kind="ExternalInput").ap()
    mkb_d = nc.dram_tensor("mkb", [S, S], I16, kind="ExternalInput").ap()
    mki_d = nc.dram_tensor("mki", [S, S], I16, kind="ExternalInput").ap()
    wqk_d = nc.dram_tensor("wqk", [F, NPAIR * 256], FP16, kind="ExternalInput").ap()
    wv_d = nc.dram_tensor("wv", [F, HPC * 64], FP16, kind="ExternalInput").ap()
    bq_d = nc.dram_tensor("bq", [128, NPAIR], F32, kind="ExternalInput").ap()
    yt_d = nc.dram_tensor("yt", [HPC, 65, S], F32, kind="ExternalOutput").ap()
    junk_d = nc.dram_tensor("junk", [64, 512], F32)  # warmup sink (Internal)

    with ExitStack() as ctx:
        tc = ctx.enter_context(tile.TileContext(nc))
        const = ctx.enter_context(tc.tile_pool(name="const", bufs=1))

        wqk_sb = const.tile([128, NF, NPAIR * 256], FP16)
        nc.sync.dma_start(wqk_sb[:], wqk_d.rearrange("(c p) n -> p c n", p=128))
        bq_sb = const.tile([128, NPAIR], F32)
        nc.sync.dma_start(bq_sb[:], bq_d)

        xt_sb = const.tile([128, NF, S], FP16)
        xt_r = xt_d.rearrange("(c p) s -> p c s", p=128)
        wv_sb = const.tile([128, NF, HPC * 64], FP16)
        for sh in range(2):
            hsl = slice(sh * (S // 2), (sh + 1) * (S // 2))
            for c in range(NF):
                nc.sync.dma_start(xt_sb[:, c, hsl], xt_r[:, c, hsl])
            if sh == 0:
                nc.sync.dma_start(
                    wv_sb[:], wv_d.rearrange("(c p) n -> p c n", p=128)
                )
        mkb_sb = const.tile([128, NT, S], I16)
        mki_sb = const.tile([128, NT, S], I16)
        mkb_r = mkb_d.rearrange("(t p) s -> p t s", p=128)
        mki_r = mki_d.rearrange("(t p) s -> p t s", p=128)
        for i in range(8):
            g = NT // 8
            sl = slice(i * g, (i + 1) * g)
            nc.sync.dma_start(mkb_sb[:, sl, :], mkb_r[:, sl, :])
            nc.sync.dma_start(mki_sb[:, sl, :], mki_r[:, sl, :])

        # V_aug in [t, 65] layout per (head, t-block); col 64 = ones
        v_sb = const.tile([128, HPC, NT, 65], BF16)
        nc.vector.memset(v_sb[:, :, :, 64:65], 1.0)
        qt_sb = [const.tile([128, S], FP16, name=f"qt{m}") for m in range(NPAIR)]
        kt_sb = [const.tile([128, S], FP16, name=f"kt{m}") for m in range(NPAIR)]

        e_pool = ctx.enter_context(tc.tile_pool(name="e", bufs=4))
        y_pool = ctx.enter_context(tc.tile_pool(name="y", bufs=2))
        ps = ctx.enter_context(tc.tile_pool(name="ps", bufs=1, space="PSUM"))

        # --- PE warmup: open the HAM clock gate (~4us of matmuls) while
        # the input DMAs land.
        NWU = 12
        wu = const.tile([128, 512], BF16)
        nc.vector.memset(wu[:], 0.0)
        pw = ps.tile([128, 512], F32, tag="pp", bufs=2, name="pw")
        for i in range(NWU):
            nc.tensor.matmul(
                pw[:], wu[:, 0:128], wu[:], start=(i == 0), stop=(i == NWU - 1)
            )
        wu_out = const.tile([64, 512], F32)
        nc.vector.tensor_copy(wu_out[:], pw[0:64, :])
        nc.sync.dma_start(junk_d.ap(), wu_out[:])

        # --- Phase 0a: QK projection (double-buffered "pp" psum tag).
        for m in range(NPAIR):
            for sq in range(S // 512):
                ssl = slice(sq * 512, (sq + 1) * 512)
                for kind in range(2):  # 0 = q, 1 = k
                    wsl = slice(m * 256 + kind * 128, m * 256 + (kind + 1) * 128)
                    pp = ps.tile([128, 512], F32, tag="pp", bufs=2, name="pp")
                    for c in range(NF):
                        nc.tensor.matmul(
                            pp[:],
                            wqk_sb[:, c, wsl],
                            xt_sb[:, c, ssl],
                            start=(c == 0),
                            stop=(c == NF - 1),
                        )
                    if kind == 0:
                        nc.vector.tensor_scalar(
                            qt_sb[m][:, ssl],
                            pp[:],
                            bq_sb[:, m : m + 1],
                            None,
                            op0=ALU.add,
                        )
                    else:
                        nc.scalar.activation(kt_sb[m][:, ssl], pp[:], AF.Copy)

        # --- Phase 0b: V via X^T-stationary, row-split; [t, d] layout.
        for tb in range(NT):
            tsl = slice(tb * 128, (tb + 1) * 128)
            vp = ps.tile([128, 512], F32, tag="pp", bufs=2, name="vp")
            for c in range(NF):
                nc.tensor.matmul(
                    vp[:, 0 : HPC * 64],
                    xt_sb[:, c, tsl],
                    wv_sb[:, c, :],
                    start=(c == 0),
                    stop=(c == NF - 1),
                )
            # [128, 4, 64] strided dst: per-head 64-col V blocks
            nc.scalar.activation(
                v_sb[:, :, tb, 0:64], vp[:, 0 : HPC * 64], AF.Copy
            )

        # --- Attention: per (pair, query-block of 512), t-blocks taken
        # in PAIRS.  Per t-block-pair p two per-head psum tiles
        # [128, 1024] = [tb=2p | tb=2p+1]; the k=64 row-tile scores MMs
        # for h0/h1 stay adjacent and overlap in the PE array, and all
        # elementwise ops run at N=1024 efficiency.
        #   h0 -> ACT exp then DVE mask-mult; h1 -> fused single DVE op
        # (Schraudolph exp+mask), except H1_ACT_PAIRS on the ACT path
        # (engine balance).  Software pipeline emits attn*V(p-1) BEFORE
        # scores(p+1) so the PE FIFO never blocks on the exp latency
        # (keeps the HAM clock-gate open).
        H1_ACT_PAIRS = frozenset((1, 4, 6))
        NP2 = NT // 2  # 8 t-block pairs
        for m in range(NPAIR):
            qt, kt = qt_sb[m], kt_sb[m]
            for qb in range(4):
                qsl = slice(qb * 512, (qb + 1) * 512)
                py = [
                    ps.tile([65, 512], F32, tag=t, name=f"py_{t}")
                    for t in ("ya", "yb")
                ]
                sc, es = {}, {}

                def emit_scores(p):
                    t0 = ps.tile([128, 1024], F32, tag="s0", bufs=1, name="s0")
                    t1 = ps.tile([128, 1024], F32, tag="s1", bufs=1, name="s1")
                    for i in range(2):
                        tsl = slice((2 * p + i) * 128, (2 * p + i + 1) * 128)
                        isl = slice(i * 512, (i + 1) * 512)
                        nc.tensor.matmul(
                            t0[:, isl], kt[0:64, tsl], qt[0:64, qsl],
                            start=True, stop=True,
                        )
                        nc.tensor.matmul(
                            t1[:, isl], kt[64:128, tsl], qt[64:128, qsl],
                            start=True, stop=True,
                        )
                    sc[p] = (t0, t1)

                def emit_exp(p):
                    t0, t1 = sc.pop(p)
                    e0 = e_pool.tile([128, 1024], BF16, tag="e0", name="e0")
                    e1 = e_pool.tile([128, 1024], BF16, tag="e1", name="e1")
                    nc.scalar.activation(
                        e0[:], t0[:], AF.Exp, scale=1.0 / A_SCALE
                    )
                    if p in H1_ACT_PAIRS:
                        nc.scalar.activation(
                            e1[:], t1[:], AF.Exp, scale=1.0 / A_SCALE
                        )
                    else:
                        nc.vector.tensor_tensor(
                            e1[:].bitcast(I16),
                            t1[:],
                            mki_sb[:, 2 * p : 2 * p + 2, qsl],
                            op=ALU.add,
                        )
                    es[p] = (e0, e1)

                def emit_mask(p):
                    e0, e1 = es[p]
                    nc.vector.tensor_tensor(
                        e0[:],
                        e0[:],
                        mkb_sb[:, 2 * p : 2 * p + 2, qsl].bitcast(BF16),
                        op=ALU.mult,
                    )
                    if p in H1_ACT_PAIRS:
                        nc.vector.tensor_tensor(
                            e1[:],
                            e1[:],
                            mkb_sb[:, 2 * p : 2 * p + 2, qsl].bitcast(BF16),
                            op=ALU.mult,
                        )

                def emit_av(p):
                    e0, e1 = es.pop(p)
                    for j, e in ((0, e0), (1, e1)):
                        for i in range(2):
                            tb = 2 * p + i
                            nc.tensor.matmul(
                                py[j][:],
                                v_sb[:, 2 * m + j, tb, :],
                                e[:, i * 512 : (i + 1) * 512],
                                start=(tb == 0),
                                stop=(tb == NT - 1),
                            )

                emit_scores(0)
                emit_exp(0)
                for p in range(NP2):
                    if p >= 1:
                        emit_av(p - 1)
                    if p + 1 < NP2:
                        emit_scores(p + 1)
                        emit_exp(p + 1)
                    emit_mask(p)
                emit_av(NP2 - 1)
                for j in range(2):
                    y_sb = y_pool.tile([65, 512], F32, tag=f"y{j}", name="y_sb")
                    nc.scalar.activation(y_sb[:], py[j][:], AF.Copy)
                    nc.sync.dma_start(
                        yt_d[2 * m + j, :, qsl], y_sb[:]
                    )

    nc.compile()
    return nc


_NC_CACHE = {}


def _get_nc():
    if "nc" not in _NC_CACHE:
        _NC_CACHE["nc"] = build_nc()
    return _NC_CACHE["nc"]


def make_in_maps(X, A, W, b):
    X = np.ascontiguousarray(np.asarray(X), dtype=np.float32)
    A = np.asarray(A)
    W = np.ascontiguousarray(np.asarray(W), dtype=np.float32)
    b = np.ascontiguousarray(np.asarray(b), dtype=np.float32)
    _NC_CACHE["b"] = b
    alpha = np.float32(np.sqrt(A_SCALE / np.sqrt(np.float32(H))))
    d = np.arange(D)

    xts = [np.ascontiguousarray(X[bb].T).astype(np.float16) for bb in range(B)]
    one_bits = np.float32(1.0).astype(ml_dtypes.bfloat16).view(np.int16)
    mkbs, mkis = [], []
    for bb in range(B):
        mT = np.ascontiguousarray(A[bb].T)
        mkbs.append(np.where(mT, one_bits, np.int16(0)).astype(np.int16))
        mkis.append(
            np.where(mT, np.int16(C_UNMASK), np.int16(C_MASK)).astype(np.int16)
        )

    packs = []
    for hg in range(2):
        wqk = np.empty((F, NPAIR * 256), np.float32)
        wv = np.empty((F, HPC * 64), np.float32)
        bq = np.empty((128, NPAIR), np.float32)
        for m in range(NPAIR):
            for half in range(2):
                h = hg * HPC + 2 * m + half
                qc = d * 24 + h
                kc = d * 24 + 8 + h
                vc = d * 24 + 16 + h
                c0 = m * 256 + half * 64
                wqk[:, c0 : c0 + 64] = W[:, qc] * alpha
                wqk[:, c0 + 128 : c0 + 192] = W[:, kc] * alpha
                wv[:, (2 * m + half) * 64 : (2 * m + half + 1) * 64] = W[:, vc]
                rsl = slice(64 * half, 64 * (half + 1))
                bq[rsl, m] = b[qc] * alpha
        packs.append((wqk.astype(np.float16), wv.astype(np.float16), bq))

    in_maps = []
    for c in range(N_CORES):
        bb = c % B
        hg = c // B
        wqk, wv, bq = packs[hg]
        in_maps.append(
            {
                "xt": xts[bb],
                "mkb": mkbs[bb],
                "mki": mkis[bb],
                "wqk": wqk,
                "wv": wv,
                "bq": bq,
            }
        )
    return in_maps


def assemble_output(results):
    b_full = _NC_CACHE["b"]
    Y = np.empty((B, S, D * H), np.float32)
    Yv = Y.reshape(B, S, D, H)
    d = np.arange(D)
    for c in range(N_CORES):
        bb = c % B
        hg = c // B
        yt = results[c]["yt"]  # [HPC, 65, S]
        for j in range(HPC):
            h = hg * HPC + j
            bv = b_full[d * 24 + 16 + h]
            Yv[bb, :, :, h] = (yt[j, 0:64, :] / yt[j, 64:65, :]).T + bv[None, :]
    return Y


def kernel(X, A, W, b):
    nc = _get_nc()
    in_maps = make_in_maps(X, A, W, b)
    res = bass_utils.run_bass_kernel_spmd(
        nc, in_maps, core_ids=list(range(N_CORES))
    ).results
    return assemble_output(res)
